# revision 48
# baseline (speedup 1.0000x reference)
"""nn_DirAttention kernel for 8 Trainium2 NeuronCores.

Strategy: data-parallel over batch (B=8, one batch element per core).
Per core, the directional attention

    ah[o,i,j] = sum_k Wc[o,k] * Qh[k,i] * Kh[k,j]   (k = C*L = 4096)

is computed by materialising G[k,(j,i)] = Kh[k,j]*Qh[k,i] per 128-row
k-block on the Vector engine (outer-product broadcast via a
column-duplicated K so every operand presents dense bf16 pairs to the
DVE -> 2x mode), then accumulating ah = Wc' @ G on the PE with even/odd
k-blocks on the two halves of the array.  Softmax over the channel
(partition) axis uses an ACT exp with per-partition bias bc, a
ones-matmul for the column sums, a 64-lane reciprocal via a DRAM
shuffle, and a DMA partition-broadcast of 1/Z.  The 3x3 conv runs as
shifted accumulating matmuls over zero-padded SBUF images, with both
image halves accumulating into one PSUM pass per output chunk.
BatchNorm is folded into the conv weights on the host.

The projections read a single x tile with parity-offset access
patterns (no materialised shifted copies), the Z sums contract K=64,
the conv's second image pass is K=64 (no zero rows), the BN shift
runs on the ACT engine, and y is written back in bf16.

Schedule notes (the big wins over the first working version):
- All memsets ride Pool/idle engines, never the in-order DVE queue
  (the catB ones-row memset alone blocked the first G mul by ~4us).
- The final (h/jh1) chain's softmax denominators skip the DRAM
  round-trip: Z matmuls with a ones[64,32] lhsT replicate each
  512-col chunk's sums across a psum band, one ACT Ln (fp16) +
  (-1)-weight broadcast matmuls put -lnZ on the hat partitions, and
  ACT exp yields 1/Z in bf16 SBUF for 2x-mode hat muls.  exp/ln/
  identity/copy are pinned to one ACT table (_patch_act_tables) so
  no table reloads land on the tail critical path.
- conv_A rp3 and the left 31 output columns of the conv's catB pass
  (which only need h/jh0's attention) interleave into the h/jh1 G
  stream; only 33 columns per row-pair remain in the tail.
- gpool bufs=16 rides out the PE's ah-psum wait at chain
  transitions, so the DVE G stream never stalls mid-kernel.
- G muls measure ~1.14us/[128,2048] = the DVE 2x_1p floor; the G
  stream is DVE-saturated end to end, which is the kernel's binding
  constraint (~157us of ~196us total).
"""

import sys

for _p in ("/opt/trn_rl_repo",):
    if _p not in sys.path:
        sys.path.append(_p)

import numpy as np
import ml_dtypes

import concourse.bacc as bacc
import concourse.bass as bass
import concourse.mybir as mybir
import concourse.tile as tile
from concourse.bass_utils import run_bass_kernel_spmd

BF16 = mybir.dt.bfloat16
F16 = mybir.dt.float16
F32 = mybir.dt.float32
B, C, L = 8, 64, 64
N = L * L  # 4096
NKB = 32  # 128-row k-blocks in the C*L contraction
BN_EPS = 1e-5
PAD = L + 2  # 66, padded row stride for the conv images

_CACHE = {}


def _patch_act_tables(arch):
    """Pin exp/ln/identity/copy activations to natural_log_exp_and_others.

    The table-load inserter picks, per activation, the first act-func-set
    containing its function; exp -> exp_and_others but ln -> a different
    set, which puts two ~1.3us ACT table reloads on the tail critical
    path.  natural_log_exp_and_others contains all four functions this
    kernel uses, so stripping them from every other set (set ORDER is
    untouched -- act_func_set_id is positional) makes the inserter place
    a single load and never swap.
    """
    import concourse.hw_specs as hw_specs
    tabs = hw_specs.get_activation_tables(arch)  # functools.cache -> shared
    funcs = {mybir.ActivationFunctionType.Exp,
             mybir.ActivationFunctionType.Ln,
             mybir.ActivationFunctionType.Identity,
             mybir.ActivationFunctionType.Copy}
    if funcs - tabs.get("natural_log_exp_and_others", set()):
        return  # unexpected act_info layout; leave the default behaviour
    for name, s in tabs.items():
        if name != "natural_log_exp_and_others":
            s.difference_update(funcs)


def _build_nc(debug=False):
    nc = bacc.Bacc(target_bir_lowering=False)
    _patch_act_tables(nc.m.arch)

    # ---- DRAM parameters -------------------------------------------------
    x2bf = nc.dram_tensor("x2bf", [128, N // 2], BF16, kind="ExternalInput")
    wqk_d = nc.dram_tensor("wqk", [128, 128], BF16, kind="ExternalInput")
    wcpt = nc.dram_tensor("wcpt", [128, NKB, 64], BF16, kind="ExternalInput")
    selneg_d = nc.dram_tensor("selneg", [128, 4, 64], F16, kind="ExternalInput")
    woa = nc.dram_tensor("woa", [128, 9, 64], BF16, kind="ExternalInput")
    wob = nc.dram_tensor("wob", [65, 9, 64], BF16, kind="ExternalInput")
    bias3_d = nc.dram_tensor("bias3", [128, 3], F32, kind="ExternalInput")
    dv_d = nc.dram_tensor("d_vec", [64, 1], F32, kind="ExternalInput")
    ident_d = nc.dram_tensor("ident", [128, 64], BF16, kind="ExternalInput")
    y = nc.dram_tensor("y", [C, N], BF16, kind="ExternalOutput")
    taps = {}
    if debug:
        for nm, shp, dt in [
            ("t_att", [64, N], F32), ("t_z", [64, N], F32),
            ("t_hatt", [64, N], F32), ("t_watt", [64, N], F32),
        ]:
            taps[nm] = nc.dram_tensor(nm, shp, dt, kind="ExternalOutput")

    from contextlib import ExitStack
    with tile.TileContext(nc) as tc, ExitStack() as _es:
        consts = _es.enter_context(tc.tile_pool(name="consts", bufs=1))
        qk = _es.enter_context(tc.tile_pool(name="qk", bufs=1))
        work = _es.enter_context(tc.tile_pool(name="work", bufs=2))
        gpool = _es.enter_context(tc.tile_pool(name="gpool", bufs=16))
        dpool = _es.enter_context(tc.tile_pool(name="dscratch", bufs=2, space="DRAM"))

        # ---- constant loads ---------------------------------------------
        xs = consts.tile([64, N], BF16)     # x image for apply/conv stages
        x2 = consts.tile([128, N // 2], BF16)  # x col-halves stacked: fast load
        wqk_sb = consts.tile([128, 128], BF16)
        wc_sb = consts.tile([128, NKB, 64], BF16)
        woa_sb = consts.tile([128, 9, 64], BF16)
        wob_sb = consts.tile([65, 9, 64], BF16)
        bias3 = consts.tile([128, 3], F32)
        dvv2 = consts.tile([128, 1], F32)
        ones = consts.tile([128, 1], BF16)
        ones32 = consts.tile([64, 32], BF16)
        selneg = consts.tile([128, 4, 64], F16)
        ident_sb = consts.tile([128, 64], BF16)
        warm = consts.tile([128, 512], BF16)
        bqq, bkk, bcv = bias3[:, 0:1], bias3[:, 1:2], bias3[0:64, 2:3]

        # DMA rate is ~per-partition-bytes and concurrent transfers on one
        # queue share it, so the proj-gating x2 halves each get a queue to
        # themselves; everything else rides the scalar queue in fine
        # chunks (the attention matmuls consume wc k-blocks in order).
        # xs is built FROM x2 by on-chip DMAs -- dependency-gated, so it
        # cannot compete with the x2 loads.
        # NOTE: the scalar queue carries ONLY the small early loads -- DMA
        # descriptor issuance on it would otherwise serialize ahead of the
        # projection ACT copies that gate the first G tiles.
        # warm memset on the DVE (idle until the first G mul anyway): the
        # PE warm matmuls (clock ramp before the projections) gate on it.
        nc.vector.memset(warm[:], 1.0)
        # the gpsimd queue finishes boot ~1us before sync, so the two
        # projection-gating loads (x2 first half + wqk) issue there; the
        # scalar queue carries NO dma issues (its head holds the ~1.3us
        # ACT table load, which would delay the q/kd projection writes)
        nc.gpsimd.dma_start(out=x2[:, 0:1024], in_=x2bf[:, 0:1024])
        nc.gpsimd.dma_start(out=wqk_sb[:], in_=wqk_d[:])
        nc.sync.dma_start(out=x2[:, 1024:2048], in_=x2bf[:, 1024:2048])
        nc.sync.dma_start(out=bias3[:], in_=bias3_d[:])
        # the scalar queue carries NO dma issues: q/kd projection writes
        # start right after the ACT table load this way
        nc.gpsimd.dma_start(out=wc_sb[:, 0:4, :], in_=wcpt[:, 0:4, :])
        nc.sync.dma_start(out=xs[:, 0:2048], in_=x2[0:64, :])
        nc.gpsimd.dma_start(out=xs[:, 2048:4096], in_=x2[64:128, :])
        for wch in range(1, 4):
            nc.sync.dma_start(out=wc_sb[:, 4 * wch:4 * (wch + 1), :],
                              in_=wcpt[:, 4 * wch:4 * (wch + 1), :])
        for wch in range(4, 8):
            nc.gpsimd.dma_start(out=wc_sb[:, 4 * wch:4 * (wch + 1), :],
                                in_=wcpt[:, 4 * wch:4 * (wch + 1), :])
        nc.sync.dma_start(out=woa_sb[:], in_=woa[:])
        nc.gpsimd.dma_start(out=wob_sb[:], in_=wob[:])
        nc.gpsimd.dma_start(out=dvv2[0:64], in_=dv_d[:])
        nc.gpsimd.dma_start(out=dvv2[64:128], in_=dv_d[:])
        nc.gpsimd.dma_start(out=ident_sb[:], in_=ident_d[:])
        nc.gpsimd.dma_start(out=selneg[:], in_=selneg_d[:])
        # memsets ride the (otherwise idle) Pool engine: keeping them off
        # the in-order DVE queue lets the first G mul start ~6us earlier
        nc.gpsimd.memset(ones[:], 1.0)
        nc.gpsimd.memset(ones32[:], 1.0)

        # conv image buffers (zero ring borders; interiors fully written)
        catA = consts.tile([128, PAD * PAD], BF16)  # rows 0-63 x, 64-127 w_att
        # catB row 64 is all-ones: with a matching conv weight row it adds
        # relu-bias + BN-shift into the conv psum, so the epilogue is a
        # single per-partition max against the BN shift.
        catB = consts.tile([65, PAD * PAD], BF16)   # h_att + ones row
        for t, p in ((catA, 128), (catB, 64)):
            base = t[0:p, :]
            nc.gpsimd.memset(base[:, 0:PAD], 0.0)
            nc.gpsimd.memset(base[:, (PAD - 1) * PAD:PAD * PAD], 0.0)
            nc.gpsimd.memset(
                bass.AP(tensor=base.tensor, offset=base.offset + PAD,
                        ap=[base.ap[0], [PAD, L]]), 0.0)
            nc.gpsimd.memset(
                bass.AP(tensor=base.tensor, offset=base.offset + PAD + L + 1,
                        ap=[base.ap[0], [PAD, L]]), 0.0)
        nc.gpsimd.memset(catB[64:65, :], 1.0)

        def pad_interior_ap(t, p0, p1, row0=0, nrows=L):
            base = t[p0:p1, :]
            return bass.AP(tensor=base.tensor,
                           offset=base.offset + (row0 + 1) * PAD + 1,
                           ap=[base.ap[0], [PAD, nrows], [1, L]])

        # x part of the conv image
        nc.sync.dma_start(out=pad_interior_ap(catA, 0, 64), in_=xs[:])

        # ---- projections -------------------------------------------------
        # Per direction: Q [128, 32, 64] (block kb = spatial pair, partition
        # = (parity, channel)), Kdup [128, 32, 64, 2] (K duplicated pairs).
        # The parity halves read the single x tile at +1 / +64 offsets.
        q_t = {d: qk.tile([128, NKB, 64], BF16, tag=f"q{d}", name=f"q_{d}") for d in "hw"}
        kd_t = {d: qk.tile([128, NKB, 64, 2], BF16, tag=f"k{d}", name=f"kd_{d}") for d in "hw"}

        with tc.tile_pool(name="projps", bufs=6, space="PSUM") as pps, \
             tc.tile_pool(name="warmps", bufs=1, space="PSUM") as wps:
            # clock-warming dummy matmuls (PE otherwise idles until x lands)
            wt = wps.tile([1, 512], F32, tag="warm")
            for _ in range(4):
                nc.tensor.matmul(out=wt[:], lhsT=warm[:, 0:1], rhs=warm[:],
                                 start=True, stop=True)
            # w first: its projections read the fast-loading stacked x2
            # image ((h<32, h>=32) on partition halves, clean t8-level
            # split).  h's projections read the slowly-arriving xs and run
            # during w's G phase, well off the critical path.
            for d in "wh":
                for t8 in ((0, 2, 1, 3) if d == "w" else range(4)):
                    for proj in "qk":
                        wcol = 0 if proj == "q" else 64
                        bias = bqq if proj == "q" else bkk
                        ps = pps.tile([128, 8, 64], F32, tag="proj")
                        for half in range(2):  # 4 g per matmul
                            g0 = t8 * 8 + half * 4
                            for par in range(2):
                                if d == "h":
                                    rhs = bass.AP(
                                        tensor=xs.tensor,
                                        offset=xs.offset + 2 * g0 + par,
                                        ap=[xs.ap[0], [2, 4], [64, 64]])
                                    nc.tensor.matmul(
                                        out=ps[par * 64:(par + 1) * 64,
                                               half * 4:(half + 1) * 4, :],
                                        lhsT=wqk_sb[0:64, wcol:wcol + 64],
                                        rhs=rhs,
                                        start=True, stop=True,
                                        skip_group_check=True,
                                        tile_position=(0, par * 64))
                                    continue
                                hi = t8 >= 2
                                p0 = 64 * hi
                                xb = x2[p0:p0 + 64, :]
                                rhs = bass.AP(
                                    tensor=xb.tensor,
                                    offset=xb.offset + 128 * g0 + 64 * par
                                    - hi * 2048,
                                    ap=[xb.ap[0], [128, 4], [1, 64]])
                                nc.tensor.matmul(
                                    out=ps[par * 64:(par + 1) * 64,
                                           half * 4:(half + 1) * 4, :],
                                    lhsT=wqk_sb[p0:p0 + 64, wcol:wcol + 64],
                                    rhs=rhs,
                                    start=True, stop=True,
                                    skip_group_check=True,
                                    tile_position=(p0, par * 64))
                        if proj == "q":
                            nc.scalar.activation(
                                out=q_t[d][:, t8 * 8:(t8 + 1) * 8, :], in_=ps[:],
                                func=mybir.ActivationFunctionType.Identity,
                                bias=bias[:], scale=1.0)
                        else:
                            for dup in range(2):
                                dst = bass.AP(
                                    tensor=kd_t[d].tensor,
                                    offset=kd_t[d].offset + t8 * 8 * 128 + dup,
                                    ap=[kd_t[d].ap[0], [128, 8], [2, 64]])
                                nc.scalar.activation(
                                    out=dst, in_=ps[:],
                                    func=mybir.ActivationFunctionType.Identity,
                                    bias=bias[:], scale=1.0)

        # ---- attention + softmax + apply + conv --------------------------
        att_t = {d: work.tile([64, L, L], BF16, tag=f"att{d}", bufs=1,
                              name=f"att_{d}") for d in "hw"}
        hat_t = {"w": work.tile([64, N], BF16, tag="hatw", bufs=1, name="hat_w")}

        cv_tiles = {}

        def conv_A(cps, rps):
            # catA half (x + h_att): runs during the w Z-chain latency
            for rp in rps:
                cv_tiles[rp] = cv = cps.tile([128, 512], F32, tag="cv", name="cv")
                for tap in range(9):
                    dy, dx = tap // 3, tap % 3
                    for half in range(2):
                        r = rp * 2 + half
                        off = (r * 8 + dy) * PAD + dx
                        rhs = bass.AP(tensor=catA.tensor, offset=catA.offset + off,
                                      ap=[catA.ap[0], [PAD, 8], [1, 64]])
                        nc.tensor.matmul(out=cv[half * 64:(half + 1) * 64, :],
                                         lhsT=woa_sb[:, tap, :], rhs=rhs,
                                         start=(tap == 0), stop=False,
                                         skip_group_check=True,
                                         tile_position=(0, half * 64))

        def conv_B_taps(rp, c0, ncols, stop):
            # catB half (h_att + ones row, K=65) accumulates onto the
            # A-pass result; [c0, c0+ncols) is the output-column slice.
            # The left 31 columns only need h_att j<32 (the h0 chain), so
            # they interleave into the h1 G stream; the right 33 columns
            # run in the tail.
            cv = cv_tiles[rp]
            for tap in range(9):
                dy, dx = tap // 3, tap % 3
                for half in range(2):
                    r = rp * 2 + half
                    off = (r * 8 + dy) * PAD + dx + c0
                    rhs = bass.AP(tensor=catB.tensor, offset=catB.offset + off,
                                  ap=[catB.ap[0], [PAD, 8], [1, ncols]])
                    cvh = cv[half * 64:(half + 1) * 64, :]
                    out = bass.AP(tensor=cvh.tensor, offset=cvh.offset + c0,
                                  ap=[cvh.ap[0], [64, 8], [1, ncols]])
                    nc.tensor.matmul(out=out,
                                     lhsT=wob_sb[:, tap, :], rhs=rhs,
                                     start=False, stop=(stop and tap == 8),
                                     skip_group_check=True,
                                     tile_position=(0, half * 64))

        def conv_B_fin(rp):
            # psum already holds conv + bo + d (ones-row bias), so
            # y = relu(conv + bo) + d == max(psum, d).  Per-half maxes so
            # each y DMA issues as soon as its half is ready; 4 ysb2 bufs
            # + two queues keep the per-rp epilogues from serializing.
            cv = cv_tiles[rp]
            ysb2 = work.tile([128, 512], BF16, tag="ysb2", bufs=4, name="ysb2")
            qa, qb = (nc.sync, nc.scalar) if rp % 2 == 0 else (nc.scalar, nc.sync)
            for half, q in ((0, qa), (1, qb)):
                sl = slice(half * 64, (half + 1) * 64)
                with nc.allow_low_precision(reason="y emitted in bf16"):
                    nc.vector.tensor_scalar_max(out=ysb2[sl, :], in0=cv[sl, :],
                                                scalar1=dvv2[sl, :])
                q.dma_start(out=y[:, (2 * rp + half) * 512:
                                  (2 * rp + half + 1) * 512],
                            in_=ysb2[sl, :])

        with tc.tile_pool(name="ahps", bufs=1, space="PSUM") as aps, \
             tc.tile_pool(name="cvps", bufs=4, space="PSUM") as cps:

            def g_chain(d, jh, interleave=(), final=False, post_hs=None):
                # G production + ah accumulation for one (direction, column
                # half).  `interleave` maps kbp -> [fn] emitting deferred DVE
                # ops (reciprocals / apply muls of the PREVIOUS chain) into
                # the middle of this chain's G stream, so their input DMAs
                # have landed by the time the in-order DVE queue reaches
                # them.
                q, kd = q_t[d], kd_t[d]
                ah = aps.tile([128, 2048], F32, tag="ah", name="ah")
                inter = dict(interleave)
                for kbp in range(NKB // 2):
                    for fn in inter.get(kbp, ()):
                        fn()
                    grhs = {}
                    for half in range(2):
                        kb = kbp * 2 + half
                        g = gpool.tile([128, 32, 64], BF16, tag="g", name=f"g{half}")
                        # G[k, j, i] = K[k,j] * Q[k,i] (2x-mode paired APs)
                        in0 = bass.AP(
                            tensor=kd.tensor,
                            offset=kd.offset + kb * 128 + jh * 64,
                            ap=[kd.ap[0], [2, 32], [0, 32], [1, 2]])
                        in1 = bass.AP(
                            tensor=q.tensor, offset=q.offset + kb * 64,
                            ap=[q.ap[0], [0, 32], [2, 32], [1, 2]])
                        gout = bass.AP(
                            tensor=g.tensor, offset=g.offset,
                            ap=[g.ap[0], [64, 32], [2, 32], [1, 2]])
                        nc.vector.tensor_mul(out=gout, in0=in0, in1=in1)
                        grhs[half] = g[:].rearrange("p a b -> p (a b)")
                    for ns in range(4):
                        for half in range(2):
                            kb = kbp * 2 + half
                            nc.tensor.matmul(
                                out=ah[half * 64:half * 64 + 64,
                                       ns * 512:(ns + 1) * 512],
                                lhsT=wc_sb[:, kb, :],
                                rhs=grhs[half][:, ns * 512:(ns + 1) * 512],
                                start=(kbp == 0),
                                stop=(kbp == NKB // 2 - 1 and ns == 3),
                                skip_group_check=True,
                                tile_position=(0, half * 64))
                # fold the odd-half partial into the even-half region via an
                # identity matmul, in quarter-column slices so copy and
                # matmul pipeline.  The final chain splits the copies over
                # ACT and DVE (both free once the last G tile is out) so the
                # fold wall-time halves; mid-phase chains keep them on ACT.
                fold = work.tile([128, 2048], BF16, tag="fold", name="fold", bufs=2)
                for ns in range(4):
                    sl = slice(ns * 512, (ns + 1) * 512)
                    if final and ns % 2:
                        with nc.allow_low_precision(reason="fold in bf16"):
                            nc.vector.tensor_copy(out=fold[64:128, sl],
                                                  in_=ah[64:128, sl])
                    else:
                        nc.scalar.copy(out=fold[64:128, sl], in_=ah[64:128, sl])
                    nc.tensor.matmul(
                        out=ah[0:64, sl],
                        lhsT=ident_sb[64:128, :],
                        rhs=fold[64:128, sl],
                        start=False, stop=True,
                        skip_group_check=True,
                        tile_position=(64, 0))
                # exp with transposed read, ah[(j,i)] -> att[(i, j)], in two
                # i-halves so the Z sums of a final chain can start early.
                # post_hs emits each half's Z matmuls right after its exp:
                # consecutive ACT exps otherwise share a batched semaphore
                # and the first half's Z would wait for the second exp too.
                for hs in range(2):
                    src = bass.AP(tensor=ah.tensor, offset=ah.offset + hs * 32,
                                  ap=[[ah.ap[0][0], 64], [1, 32], [64, 32]])
                    nc.scalar.activation(
                        out=att_t[d][:, hs * 32:(hs + 1) * 32,
                                     jh * 32:(jh + 1) * 32], in_=src,
                        func=mybir.ActivationFunctionType.Exp,
                        bias=bcv[:], scale=1.0)
                    if post_hs is not None:
                        post_hs(hs)

            def z_sums(att3, chunks, zs_dst):
                # Z column sums: K=64 ones-matmuls into four disjoint
                # column-groups (psum rows 0/32/64/96) of ONE bank-wide
                # tile, then a single strided DMA spreads them into the
                # [rows, 32/64-wide] zs block for the reciprocal.
                zt4 = cps.tile([128, 512], F32, tag="cv", name="zt4")
                for c4, (off, apf) in enumerate(chunks):
                    rhs = bass.AP(tensor=att3.tensor, offset=att3.offset + off,
                                  ap=[att3.ap[0]] + apf)
                    nc.tensor.matmul(out=zt4[32 * c4:32 * c4 + 1, :],
                                     lhsT=ones[0:64], rhs=rhs,
                                     start=True, stop=True,
                                     skip_group_check=True,
                                     tile_position=(0, 32 * c4))
                zsp = work.tile([128, 512], BF16, tag="zsp", bufs=2, name="zsp")
                with nc.allow_low_precision(reason="Z sums to bf16 for 1/Z"):
                    nc.scalar.copy(out=zsp[:], in_=zt4[:])
                nc.scalar.dma_start(
                    out=zs_dst,
                    in_=bass.AP(tensor=zsp.tensor, offset=zsp.offset,
                                ap=[[zsp.ap[0][0] * 32, 4], [1, 512]]))

            def z_mms_j(d, jh):
                # per-column-half Z sums, (i-major, 32 j) layout
                zs = work.tile([64, 32], BF16, tag="zsj", bufs=2, name="zsj")
                z_sums(att_t[d],
                       [(c4 * 16 * 64 + jh * 32, [[64, 16], [1, 32]])
                        for c4 in range(4)], zs[:])
                return zs

            def rz_chain_j(zs):
                # reciprocal + DRAM-broadcast of 1/Z for one column half
                rzs = work.tile([64, 32], BF16, tag="rzsj", bufs=2, name="rzsj")
                with nc.allow_low_precision(reason="1/Z multiplier in bf16"):
                    nc.vector.reciprocal(out=rzs[:], in_=zs[:])
                rz = dpool.tile([64, 32], BF16, tag="rzdj")
                nc.scalar.dma_start(out=rz[:], in_=rzs[:])
                rzb = work.tile([64, 2048], BF16, tag="rzbj", bufs=2, name="rzbj")
                for qi, queue in enumerate((nc.sync, nc.scalar)):
                    queue.dma_start(
                        out=rzb[:, qi * 1024:(qi + 1) * 1024],
                        in_=bass.AP(tensor=rz.tensor, offset=rz.offset + qi * 1024,
                                    ap=[[0, 64], [32, 32], [1, 32]]))
                return rzb

            # ---- w direction: full-width softmax chain, deferred into the
            # h/jh0 G stream ----------------------------------------------
            g_chain("w", 0)
            g_chain("w", 1)
            att_w = att_t["w"][:].rearrange("p a b -> p (a b)")
            if debug:
                nc.sync.dma_start(out=taps["t_z"][:], in_=att_w[:])
            zs_w = work.tile([64, 64], BF16, tag="zsw", bufs=1)
            for hb in range(2):
                z_sums(att_t["w"],
                       [((hb * 4 + c4) * 512, [[1, 512]]) for c4 in range(4)],
                       zs_w[hb * 32:(hb + 1) * 32, :])
            rzb_w = work.tile([64, N], BF16, tag="rzbw", bufs=1)
            tmp_w = work.tile([64, N], BF16, tag="tmpw", bufs=1)

            def w_recip():
                rzs = work.tile([64, 64], BF16, tag="rzsw", bufs=1)
                with nc.allow_low_precision(reason="1/Z multiplier in bf16"):
                    nc.vector.reciprocal(out=rzs[:], in_=zs_w[:])
                rz = dpool.tile([64, 64], BF16, tag="rzdw")
                nc.scalar.dma_start(out=rz[:], in_=rzs[:])
                for ch in range(2):
                    sl = slice(ch * 2048, (ch + 1) * 2048)
                    nc.sync.dma_start(
                        out=rzb_w[:, sl],
                        in_=bass.AP(tensor=rz.tensor, offset=rz.offset + ch * 2048,
                                    ap=[[0, 64], [64, 32], [1, 64]]))

            def w_tmp():
                nc.vector.tensor_mul(out=tmp_w[:], in0=att_w[:], in1=xs[:])

            def w_hat(ch):
                sl = slice(ch * 2048, (ch + 1) * 2048)
                nc.vector.tensor_mul(out=hat_t["w"][:, sl], in0=tmp_w[:, sl],
                                     in1=rzb_w[:, sl])
                nc.sync.dma_start(
                    out=pad_interior_ap(catA, 64, 128, row0=ch * 32, nrows=32),
                    in_=hat_t["w"][:, sl])

            # ---- h direction, column half 0 ------------------------------
            g_chain("h", 0, {1: [w_recip], 2: [w_tmp],
                             4: [lambda: w_hat(0)], 5: [lambda: w_hat(1)]})
            zs_h0 = z_mms_j("h", 0)
            conv_A(cps, [0, 1, 2])
            conv_A3 = lambda: conv_A(cps, [3])  # noqa: E731
            rzb_h0 = [None]
            tmp_h0 = work.tile([64, 2048], BF16, tag="tmph0", bufs=1)

            def h0_recip():
                rzb_h0[0] = rz_chain_j(zs_h0)

            def h0_tmp():
                in0 = bass.AP(tensor=att_t["h"].tensor, offset=att_t["h"].offset,
                              ap=[att_t["h"].ap[0], [64, 64], [1, 32]])
                in1 = bass.AP(tensor=xs.tensor, offset=xs.offset,
                              ap=[xs.ap[0], [64, 64], [1, 32]])
                nc.vector.tensor_mul(out=tmp_h0[:], in0=in0, in1=in1)

            def h0_hat():
                cb = catB[0:64, :]
                nc.vector.tensor_mul(
                    out=bass.AP(tensor=cb.tensor, offset=cb.offset + PAD + 1,
                                ap=[cb.ap[0], [PAD, 64], [1, 32]]),
                    in0=tmp_h0[:].rearrange("p (a b) -> p a b", b=32),
                    in1=rzb_h0[0][:].rearrange("p (a b) -> p a b", b=32))

            # ---- h direction, column half 1 (the tail) -------------------
            # conv_A rp3 and the left convB column-halves (which only need
            # h0's j<32 attention, ready at slot 5) fill the PE's idle
            # cycles inside this chain's G stream, leaving just the right
            # 33 output columns per rp for the tail.
            g_chain("h", 1, {3: [h0_recip], 4: [h0_tmp], 6: [h0_hat],
                             7: [conv_A3],
                             9: [lambda: conv_B_taps(0, 0, 31, False)],
                             11: [lambda: conv_B_taps(1, 0, 31, False)],
                             13: [lambda: conv_B_taps(2, 0, 31, False)],
                             15: [lambda: conv_B_taps(3, 0, 31, False)]},
                    final=True)
            att_h = att_t["h"][:].rearrange("p a b -> p (a b)")
            if debug:
                nc.sync.dma_start(out=taps["t_att"][:], in_=att_h[:])
            # Tail softmax denominators without DRAM round-trips: Z band
            # matmuls (32-row replication via ones32) fill a [128, 512]
            # psum region -> one ACT Ln (fp16) -> per-chunk (-1)-weights
            # broadcast matmuls put -lnZ on 64 partitions -> ACT exp reads
            # psum, yielding 1/Z in bf16 SBUF for the 2x-mode hat muls.
            # All of zt4n + the four chunks pack into one psum tile (the
            # c4=3 chunk reuses the zt4n columns after Ln has read them);
            # exp/ln/identity share one ACT table (see _patch_act_tables)
            # so no table reloads appear on this chain.
            rzpt = aps.tile([128, 2048], F32, tag="ah", name="rzp")
            zt4n = rzpt[:, 0:512]
            for c4 in range(4):
                rhs = bass.AP(
                    tensor=att_t["h"].tensor,
                    offset=att_t["h"].offset + c4 * 16 * 64 + 32,
                    ap=[att_t["h"].ap[0], [64, 16], [1, 32]])
                nc.tensor.matmul(out=zt4n[32 * c4:32 * c4 + 32, :],
                                 lhsT=ones32[:], rhs=rhs,
                                 start=True, stop=True,
                                 skip_group_check=True,
                                 tile_position=(0, 32 * c4))
            lnz = work.tile([128, 512], F16, tag="lnz", bufs=1, name="lnz")
            with nc.allow_low_precision(reason="lnZ in fp16 (10-bit mantissa)"):
                nc.scalar.activation(out=lnz[:], in_=zt4n,
                                     func=mybir.ActivationFunctionType.Ln,
                                     scale=1.0)
            # all broadcasts BEFORE any exp, packed as column-tile pairs
            # at (0,0)/(0,64).  NOTE: interleaving each exp right after its
            # broadcast looks attractive but RACES on hardware
            # (nondeterministic 5e-2..9e-2 errors) -- keep the batch order.
            dsts = []
            for c4 in range(4):
                dst = rzpt[(c4 % 2) * 64:(c4 % 2) * 64 + 64,
                           (1 + c4 // 2) * 512:(2 + c4 // 2) * 512]
                nc.tensor.matmul(out=dst, lhsT=selneg[:, c4, :], rhs=lnz[:],
                                 start=True, stop=True,
                                 skip_group_check=True,
                                 tile_position=(0, (c4 % 2) * 64))
                dsts.append(dst)
            rzb4 = []
            for c4 in range(4):
                rb = work.tile([64, 512], BF16, tag="rzb4", bufs=4, name="rzb4")
                with nc.allow_low_precision(reason="1/Z multiplier in bf16"):
                    nc.scalar.activation(
                        out=rb[:], in_=dsts[c4],
                        func=mybir.ActivationFunctionType.Exp, scale=1.0)
                rzb4.append(rb)
            # apply in 4 row bands (att*x products first -- they only need
            # the exp -- then the 1/Z muls as the broadcast lands); conv
            # row-pair rp needs image rows up to 16(rp+1)+1, so emit conv
            # rp-1 after each band.
            # att*x products as ONE DVE op (the hats are rb-gated ~3us
            # later, so the coarser granularity costs nothing and saves
            # three instruction + semaphore overheads on the DVE queue)
            tw4 = work.tile([64, 2048], BF16, tag="tmph1", bufs=1, name="tmph1")
            in0 = bass.AP(tensor=att_t["h"].tensor,
                          offset=att_t["h"].offset + 32,
                          ap=[att_t["h"].ap[0], [64, 64], [1, 32]])
            in1 = bass.AP(tensor=xs.tensor, offset=xs.offset + 32,
                          ap=[xs.ap[0], [64, 64], [1, 32]])
            nc.vector.tensor_mul(out=tw4[:], in0=in0, in1=in1)
            for ch in range(4):
                cb = catB[0:64, :]
                nc.vector.tensor_mul(
                    out=bass.AP(tensor=cb.tensor,
                                offset=cb.offset + (ch * 16 + 1) * PAD + 33,
                                ap=[cb.ap[0], [PAD, 16], [1, 32]]),
                    in0=bass.AP(tensor=tw4.tensor,
                                offset=tw4.offset + ch * 512,
                                ap=[tw4.ap[0], [32, 16], [1, 32]]),
                    in1=rzb4[ch][:].rearrange("p (a b) -> p a b", b=32))
                if ch >= 1:
                    conv_B_taps(ch - 1, 31, 33, True)
                    conv_B_fin(ch - 1)
            conv_B_taps(3, 31, 33, True)
            conv_B_fin(3)

        if debug:
            nc.sync.dma_start(out=taps["t_watt"][:], in_=hat_t["w"][:])
            nc.sync.dma_start(out=taps["t_hatt"][:],
                              in_=pad_interior_ap(catB, 0, 64))

    nc.finalize()
    return nc


def _host_prep(Wq, bq, Wk, bk, Wc, bc, Wo, bo, gamma, beta, run_mean, run_var):
    bf = ml_dtypes.bfloat16
    # Wc permuted so the contraction index is (spatial, channel)
    wcp = Wc.reshape(C, C, L).transpose(0, 2, 1).reshape(C, C * L)
    wcpt = np.ascontiguousarray(
        wcp.T.reshape(NKB, 128, 64).transpose(1, 0, 2))  # [128, 32, 64]
    inv = gamma / np.sqrt(run_var + BN_EPS)
    wo_eff = Wo * inv[:, None, None, None]
    wot = wo_eff.transpose(1, 2, 3, 0).reshape(3 * C, 9, C)  # [192, 9, 64]
    # conv image A carries [x; w_att], image B carries h_att plus a ones
    # row whose tap-0 weight injects relu-bias + BN-shift into the psum
    d_vec = beta - run_mean * inv
    brow = np.zeros((1, 9, C), np.float32)
    brow[0, 0, :] = bo * inv + d_vec
    wq2 = np.concatenate([Wq.T, Wq.T])  # [128, 64]
    wk2 = np.concatenate([Wk.T, Wk.T])
    bias3 = np.stack([np.concatenate([bq, bq]), np.concatenate([bk, bk]),
                      np.concatenate([bc, bc])], axis=1)  # [128, 3]
    # -lnZ broadcast weights: selneg[:, c4, :] is -1 at partition 32*c4
    selneg = np.zeros((128, 4, 64), np.float16)
    for c4 in range(4):
        selneg[32 * c4, c4, :] = -1.0
    return {
        "selneg": selneg,
        "wqk": np.ascontiguousarray(
            np.concatenate([wq2, wk2], axis=1)).astype(bf),
        "wcpt": wcpt.astype(bf),
        "woa": np.ascontiguousarray(
            np.concatenate([wot[0:64], wot[128:192]])).astype(bf),
        "wob": np.ascontiguousarray(
            np.concatenate([wot[64:128], brow])).astype(bf),
        "bias3": np.ascontiguousarray(bias3).astype(np.float32),
        "d_vec": d_vec.reshape(64, 1).astype(np.float32),
        "ident": np.concatenate([np.zeros((64, 64), np.float32),
                                 np.eye(64, dtype=np.float32)]).astype(bf),
    }


def kernel(x, Wq, bq, Wk, bk, Wc, bc, Wo, bo, gamma, beta, run_mean, run_var,
           debug=False, trace=False, trace_kwargs=None):
    x = np.asarray(x, np.float32)
    weights = _host_prep(
        np.asarray(Wq, np.float32), np.asarray(bq, np.float32),
        np.asarray(Wk, np.float32), np.asarray(bk, np.float32),
        np.asarray(Wc, np.float32), np.asarray(bc, np.float32),
        np.asarray(Wo, np.float32), np.asarray(bo, np.float32),
        np.asarray(gamma, np.float32), np.asarray(beta, np.float32),
        np.asarray(run_mean, np.float32), np.asarray(run_var, np.float32))
    key = bool(debug)
    if key not in _CACHE:
        _CACHE[key] = _build_nc(debug=debug)
    nc = _CACHE[key]
    bf = ml_dtypes.bfloat16
    in_maps = []
    for b in range(B):
        m = dict(weights)
        xr = x[b].reshape(C, N).astype(bf)
        m["x2bf"] = np.ascontiguousarray(
            np.concatenate([xr[:, 0:N // 2], xr[:, N // 2:]], axis=0))
        in_maps.append(m)
    kwargs = {}
    if trace:
        kwargs = dict(trace=True, trace_cores=[0], **(trace_kwargs or {}))
    res = run_bass_kernel_spmd(nc, in_maps, core_ids=list(range(B)), **kwargs)
    out = np.stack([res.results[b]["y"].astype(np.float32).reshape(C, L, L)
                    for b in range(B)])
    if debug or trace:
        return out, res
    return out



# revision 50
# speedup vs baseline: 1.0066x; 1.0066x over previous
"""nn_DirAttention kernel for 8 Trainium2 NeuronCores.

Strategy: data-parallel over batch (B=8, one batch element per core).
Per core, the directional attention

    ah[o,i,j] = sum_k Wc[o,k] * Qh[k,i] * Kh[k,j]   (k = C*L = 4096)

is computed by materialising G[k,(j,i)] = Kh[k,j]*Qh[k,i] per 128-row
k-block on the Vector engine (outer-product broadcast via a
column-duplicated K so every operand presents dense bf16 pairs to the
DVE -> 2x mode), then accumulating ah = Wc' @ G on the PE with even/odd
k-blocks on the two halves of the array.  Softmax over the channel
(partition) axis uses an ACT exp with per-partition bias bc, a
ones-matmul for the column sums, a 64-lane reciprocal via a DRAM
shuffle, and a DMA partition-broadcast of 1/Z.  The 3x3 conv runs as
shifted accumulating matmuls over zero-padded SBUF images, with both
image halves accumulating into one PSUM pass per output chunk.
BatchNorm is folded into the conv weights on the host.

The projections read a single x tile with parity-offset access
patterns (no materialised shifted copies), the Z sums contract K=64,
the conv's second image pass is K=64 (no zero rows), the BN shift
runs on the ACT engine, and y is written back in bf16.

Schedule notes (the big wins over the first working version):
- All memsets ride Pool/idle engines, never the in-order DVE queue
  (the catB ones-row memset alone blocked the first G mul by ~4us).
- The final (h/jh1) chain's softmax denominators skip the DRAM
  round-trip: Z matmuls with a ones[64,32] lhsT replicate each
  512-col chunk's sums across a psum band, one ACT Ln (fp16) +
  (-1)-weight broadcast matmuls put -lnZ on the hat partitions, and
  ACT exp yields 1/Z in bf16 SBUF for 2x-mode hat muls.  exp/ln/
  identity/copy are pinned to one ACT table (_patch_act_tables) so
  no table reloads land on the tail critical path.
- conv_A rp3 and the left 31 output columns of the conv's catB pass
  (which only need h/jh0's attention) interleave into the h/jh1 G
  stream; only 33 columns per row-pair remain in the tail.
- gpool bufs=16 rides out the PE's ah-psum wait at chain
  transitions, so the DVE G stream never stalls mid-kernel.
- G muls measure ~1.14us/[128,2048] = the DVE 2x_1p floor; the G
  stream is DVE-saturated end to end, which is the kernel's binding
  constraint (~157us of ~196us total).
"""

import sys

for _p in ("/opt/trn_rl_repo",):
    if _p not in sys.path:
        sys.path.append(_p)

import numpy as np
import ml_dtypes

import concourse.bacc as bacc
import concourse.bass as bass
import concourse.mybir as mybir
import concourse.tile as tile
from concourse.bass_utils import run_bass_kernel_spmd

BF16 = mybir.dt.bfloat16
F16 = mybir.dt.float16
F32 = mybir.dt.float32
B, C, L = 8, 64, 64
N = L * L  # 4096
NKB = 32  # 128-row k-blocks in the C*L contraction
BN_EPS = 1e-5
PAD = L + 2  # 66, padded row stride for the conv images

_CACHE = {}


def _patch_act_tables(arch):
    """Pin exp/ln/identity/copy activations to natural_log_exp_and_others.

    The table-load inserter picks, per activation, the first act-func-set
    containing its function; exp -> exp_and_others but ln -> a different
    set, which puts two ~1.3us ACT table reloads on the tail critical
    path.  natural_log_exp_and_others contains all four functions this
    kernel uses, so stripping them from every other set (set ORDER is
    untouched -- act_func_set_id is positional) makes the inserter place
    a single load and never swap.
    """
    import concourse.hw_specs as hw_specs
    tabs = hw_specs.get_activation_tables(arch)  # functools.cache -> shared
    funcs = {mybir.ActivationFunctionType.Exp,
             mybir.ActivationFunctionType.Ln,
             mybir.ActivationFunctionType.Identity,
             mybir.ActivationFunctionType.Copy}
    if funcs - tabs.get("natural_log_exp_and_others", set()):
        return  # unexpected act_info layout; leave the default behaviour
    for name, s in tabs.items():
        if name != "natural_log_exp_and_others":
            s.difference_update(funcs)


def _build_nc(debug=False):
    nc = bacc.Bacc(target_bir_lowering=False)
    _patch_act_tables(nc.m.arch)

    # ---- DRAM parameters -------------------------------------------------
    x2bf = nc.dram_tensor("x2bf", [128, N // 2], BF16, kind="ExternalInput")
    wqk_d = nc.dram_tensor("wqk", [128, 128], BF16, kind="ExternalInput")
    wcpt = nc.dram_tensor("wcpt", [128, NKB, 64], BF16, kind="ExternalInput")
    selneg_d = nc.dram_tensor("selneg", [128, 4, 64], F16, kind="ExternalInput")
    woa = nc.dram_tensor("woa", [128, 9, 64], BF16, kind="ExternalInput")
    wob = nc.dram_tensor("wob", [65, 9, 64], BF16, kind="ExternalInput")
    bias3_d = nc.dram_tensor("bias3", [128, 3], F32, kind="ExternalInput")
    dv_d = nc.dram_tensor("d_vec", [64, 1], F32, kind="ExternalInput")
    ident_d = nc.dram_tensor("ident", [128, 64], BF16, kind="ExternalInput")
    y = nc.dram_tensor("y", [C, N], BF16, kind="ExternalOutput")
    taps = {}
    if debug:
        for nm, shp, dt in [
            ("t_att", [64, N], F32), ("t_z", [64, N], F32),
            ("t_hatt", [64, N], F32), ("t_watt", [64, N], F32),
        ]:
            taps[nm] = nc.dram_tensor(nm, shp, dt, kind="ExternalOutput")

    from contextlib import ExitStack
    with tile.TileContext(nc) as tc, ExitStack() as _es:
        consts = _es.enter_context(tc.tile_pool(name="consts", bufs=1))
        qk = _es.enter_context(tc.tile_pool(name="qk", bufs=1))
        work = _es.enter_context(tc.tile_pool(name="work", bufs=2))
        gpool = _es.enter_context(tc.tile_pool(name="gpool", bufs=16))
        dpool = _es.enter_context(tc.tile_pool(name="dscratch", bufs=2, space="DRAM"))

        # ---- constant loads ---------------------------------------------
        xs = consts.tile([64, N], BF16)     # x image for apply/conv stages
        x2 = consts.tile([128, N // 2], BF16)  # x col-halves stacked: fast load
        wqk_sb = consts.tile([128, 128], BF16)
        wc_sb = consts.tile([128, NKB, 64], BF16)
        woa_sb = consts.tile([128, 9, 64], BF16)
        wob_sb = consts.tile([65, 9, 64], BF16)
        bias3 = consts.tile([128, 3], F32)
        dvv2 = consts.tile([128, 1], F32)
        ones = consts.tile([128, 1], BF16)
        ones32 = consts.tile([64, 32], BF16)
        selneg = consts.tile([128, 4, 64], F16)
        ident_sb = consts.tile([128, 64], BF16)
        warm = consts.tile([128, 512], BF16)
        bqq, bkk, bcv = bias3[:, 0:1], bias3[:, 1:2], bias3[0:64, 2:3]

        # DMA rate is ~per-partition-bytes and concurrent transfers on one
        # queue share it, so the proj-gating x2 halves each get a queue to
        # themselves; everything else rides the scalar queue in fine
        # chunks (the attention matmuls consume wc k-blocks in order).
        # xs is built FROM x2 by on-chip DMAs -- dependency-gated, so it
        # cannot compete with the x2 loads.
        # NOTE: the scalar queue carries ONLY the small early loads -- DMA
        # descriptor issuance on it would otherwise serialize ahead of the
        # projection ACT copies that gate the first G tiles.
        # warm memset on the DVE (idle until the first G mul anyway): the
        # PE warm matmuls (clock ramp before the projections) gate on it.
        nc.vector.memset(warm[:], 1.0)
        # the gpsimd queue finishes boot ~1us before sync, so the two
        # projection-gating loads (x2 first half + wqk) issue there; the
        # scalar queue carries NO dma issues (its head holds the ~1.3us
        # ACT table load, which would delay the q/kd projection writes)
        nc.gpsimd.dma_start(out=x2[:, 0:1024], in_=x2bf[:, 0:1024])
        nc.gpsimd.dma_start(out=wqk_sb[:], in_=wqk_d[:])
        nc.sync.dma_start(out=x2[:, 1024:2048], in_=x2bf[:, 1024:2048])
        nc.sync.dma_start(out=bias3[:], in_=bias3_d[:])
        # the scalar queue carries NO dma issues: q/kd projection writes
        # start right after the ACT table load this way
        nc.gpsimd.dma_start(out=wc_sb[:, 0:4, :], in_=wcpt[:, 0:4, :])
        nc.sync.dma_start(out=xs[:, 0:2048], in_=x2[0:64, :])
        nc.gpsimd.dma_start(out=xs[:, 2048:4096], in_=x2[64:128, :])
        for wch in range(1, 4):
            nc.sync.dma_start(out=wc_sb[:, 4 * wch:4 * (wch + 1), :],
                              in_=wcpt[:, 4 * wch:4 * (wch + 1), :])
        for wch in range(4, 8):
            nc.gpsimd.dma_start(out=wc_sb[:, 4 * wch:4 * (wch + 1), :],
                                in_=wcpt[:, 4 * wch:4 * (wch + 1), :])
        nc.sync.dma_start(out=woa_sb[:], in_=woa[:])
        nc.gpsimd.dma_start(out=wob_sb[:], in_=wob[:])
        nc.gpsimd.dma_start(out=dvv2[0:64], in_=dv_d[:])
        nc.gpsimd.dma_start(out=dvv2[64:128], in_=dv_d[:])
        nc.gpsimd.dma_start(out=ident_sb[:], in_=ident_d[:])
        nc.gpsimd.dma_start(out=selneg[:], in_=selneg_d[:])
        # memsets ride the (otherwise idle) Pool engine: keeping them off
        # the in-order DVE queue lets the first G mul start ~6us earlier
        nc.gpsimd.memset(ones[:], 1.0)
        nc.gpsimd.memset(ones32[:], 1.0)

        # conv image buffers (zero ring borders; interiors fully written)
        catA = consts.tile([128, PAD * PAD], BF16)  # rows 0-63 x, 64-127 w_att
        # catB row 64 is all-ones: with a matching conv weight row it adds
        # relu-bias + BN-shift into the conv psum, so the epilogue is a
        # single per-partition max against the BN shift.
        catB = consts.tile([65, PAD * PAD], BF16)   # h_att + ones row
        for t, p in ((catA, 128), (catB, 64)):
            base = t[0:p, :]
            nc.gpsimd.memset(base[:, 0:PAD], 0.0)
            nc.gpsimd.memset(base[:, (PAD - 1) * PAD:PAD * PAD], 0.0)
            nc.gpsimd.memset(
                bass.AP(tensor=base.tensor, offset=base.offset + PAD,
                        ap=[base.ap[0], [PAD, L]]), 0.0)
            nc.gpsimd.memset(
                bass.AP(tensor=base.tensor, offset=base.offset + PAD + L + 1,
                        ap=[base.ap[0], [PAD, L]]), 0.0)
        nc.gpsimd.memset(catB[64:65, :], 1.0)

        def pad_interior_ap(t, p0, p1, row0=0, nrows=L):
            base = t[p0:p1, :]
            return bass.AP(tensor=base.tensor,
                           offset=base.offset + (row0 + 1) * PAD + 1,
                           ap=[base.ap[0], [PAD, nrows], [1, L]])

        # x part of the conv image
        nc.sync.dma_start(out=pad_interior_ap(catA, 0, 64), in_=xs[:])

        # ---- projections -------------------------------------------------
        # Per direction: Q [128, 32, 64] (block kb = spatial pair, partition
        # = (parity, channel)), Kdup [128, 32, 64, 2] (K duplicated pairs).
        # The parity halves read the single x tile at +1 / +64 offsets.
        q_t = {d: qk.tile([128, NKB, 64], BF16, tag=f"q{d}", name=f"q_{d}") for d in "hw"}
        kd_t = {d: qk.tile([128, NKB, 64, 2], BF16, tag=f"k{d}", name=f"kd_{d}") for d in "hw"}

        with tc.tile_pool(name="projps", bufs=6, space="PSUM") as pps, \
             tc.tile_pool(name="warmps", bufs=1, space="PSUM") as wps:
            # clock-warming dummy matmuls (PE otherwise idles until x lands)
            wt = wps.tile([1, 512], F32, tag="warm")
            for _ in range(4):
                nc.tensor.matmul(out=wt[:], lhsT=warm[:, 0:1], rhs=warm[:],
                                 start=True, stop=True)
            # w first: its projections read the fast-loading stacked x2
            # image ((h<32, h>=32) on partition halves, clean t8-level
            # split).  h's projections read the slowly-arriving xs and run
            # during w's G phase, well off the critical path.
            for d in "wh":
                for t8 in ((0, 2, 1, 3) if d == "w" else range(4)):
                    for proj in "qk":
                        wcol = 0 if proj == "q" else 64
                        bias = bqq if proj == "q" else bkk
                        ps = pps.tile([128, 8, 64], F32, tag="proj")
                        for half in range(2):  # 4 g per matmul
                            g0 = t8 * 8 + half * 4
                            for par in range(2):
                                if d == "h":
                                    rhs = bass.AP(
                                        tensor=xs.tensor,
                                        offset=xs.offset + 2 * g0 + par,
                                        ap=[xs.ap[0], [2, 4], [64, 64]])
                                    nc.tensor.matmul(
                                        out=ps[par * 64:(par + 1) * 64,
                                               half * 4:(half + 1) * 4, :],
                                        lhsT=wqk_sb[0:64, wcol:wcol + 64],
                                        rhs=rhs,
                                        start=True, stop=True,
                                        skip_group_check=True,
                                        tile_position=(0, par * 64))
                                    continue
                                hi = t8 >= 2
                                p0 = 64 * hi
                                xb = x2[p0:p0 + 64, :]
                                rhs = bass.AP(
                                    tensor=xb.tensor,
                                    offset=xb.offset + 128 * g0 + 64 * par
                                    - hi * 2048,
                                    ap=[xb.ap[0], [128, 4], [1, 64]])
                                nc.tensor.matmul(
                                    out=ps[par * 64:(par + 1) * 64,
                                           half * 4:(half + 1) * 4, :],
                                    lhsT=wqk_sb[p0:p0 + 64, wcol:wcol + 64],
                                    rhs=rhs,
                                    start=True, stop=True,
                                    skip_group_check=True,
                                    tile_position=(p0, par * 64))
                        if proj == "q":
                            nc.scalar.activation(
                                out=q_t[d][:, t8 * 8:(t8 + 1) * 8, :], in_=ps[:],
                                func=mybir.ActivationFunctionType.Identity,
                                bias=bias[:], scale=1.0)
                        else:
                            for dup in range(2):
                                dst = bass.AP(
                                    tensor=kd_t[d].tensor,
                                    offset=kd_t[d].offset + t8 * 8 * 128 + dup,
                                    ap=[kd_t[d].ap[0], [128, 8], [2, 64]])
                                nc.scalar.activation(
                                    out=dst, in_=ps[:],
                                    func=mybir.ActivationFunctionType.Identity,
                                    bias=bias[:], scale=1.0)

        # ---- attention + softmax + apply + conv --------------------------
        att_t = {d: work.tile([64, L, L], BF16, tag=f"att{d}", bufs=1,
                              name=f"att_{d}") for d in "hw"}
        hat_t = {"w": work.tile([64, N], BF16, tag="hatw", bufs=1, name="hat_w")}

        cv_tiles = {}

        def conv_A(cps, rps):
            # catA half (x + h_att): runs during the w Z-chain latency
            for rp in rps:
                cv_tiles[rp] = cv = cps.tile([128, 512], F32, tag="cv", name="cv")
                for tap in range(9):
                    dy, dx = tap // 3, tap % 3
                    for half in range(2):
                        r = rp * 2 + half
                        off = (r * 8 + dy) * PAD + dx
                        rhs = bass.AP(tensor=catA.tensor, offset=catA.offset + off,
                                      ap=[catA.ap[0], [PAD, 8], [1, 64]])
                        nc.tensor.matmul(out=cv[half * 64:(half + 1) * 64, :],
                                         lhsT=woa_sb[:, tap, :], rhs=rhs,
                                         start=(tap == 0), stop=False,
                                         skip_group_check=True,
                                         tile_position=(0, half * 64))

        def conv_B_taps(rp, c0, ncols, stop):
            # catB half (h_att + ones row, K=65) accumulates onto the
            # A-pass result; [c0, c0+ncols) is the output-column slice.
            # The left 31 columns only need h_att j<32 (the h0 chain), so
            # they interleave into the h1 G stream; the right 33 columns
            # run in the tail.
            cv = cv_tiles[rp]
            for tap in range(9):
                dy, dx = tap // 3, tap % 3
                for half in range(2):
                    r = rp * 2 + half
                    off = (r * 8 + dy) * PAD + dx + c0
                    rhs = bass.AP(tensor=catB.tensor, offset=catB.offset + off,
                                  ap=[catB.ap[0], [PAD, 8], [1, ncols]])
                    cvh = cv[half * 64:(half + 1) * 64, :]
                    out = bass.AP(tensor=cvh.tensor, offset=cvh.offset + c0,
                                  ap=[cvh.ap[0], [64, 8], [1, ncols]])
                    nc.tensor.matmul(out=out,
                                     lhsT=wob_sb[:, tap, :], rhs=rhs,
                                     start=False, stop=(stop and tap == 8),
                                     skip_group_check=True,
                                     tile_position=(0, half * 64))

        def conv_B_fin(rp):
            # psum already holds conv + bo + d (ones-row bias), so
            # y = relu(conv + bo) + d == max(psum, d).  Per-half maxes so
            # each y DMA issues as soon as its half is ready; 4 ysb2 bufs
            # + two queues keep the per-rp epilogues from serializing.
            cv = cv_tiles[rp]
            ysb2 = work.tile([128, 512], BF16, tag="ysb2", bufs=4, name="ysb2")
            qa, qb = (nc.sync, nc.scalar) if rp % 2 == 0 else (nc.scalar, nc.sync)
            for half, q in ((0, qa), (1, qb)):
                sl = slice(half * 64, (half + 1) * 64)
                with nc.allow_low_precision(reason="y emitted in bf16"):
                    nc.vector.tensor_scalar_max(out=ysb2[sl, :], in0=cv[sl, :],
                                                scalar1=dvv2[sl, :])
                q.dma_start(out=y[:, (2 * rp + half) * 512:
                                  (2 * rp + half + 1) * 512],
                            in_=ysb2[sl, :])

        with tc.tile_pool(name="ahps", bufs=1, space="PSUM") as aps, \
             tc.tile_pool(name="cvps", bufs=4, space="PSUM") as cps:

            def g_chain(d, jh, interleave=(), final=False, post_hs=None):
                # G production + ah accumulation for one (direction, column
                # half).  `interleave` maps kbp -> [fn] emitting deferred DVE
                # ops (reciprocals / apply muls of the PREVIOUS chain) into
                # the middle of this chain's G stream, so their input DMAs
                # have landed by the time the in-order DVE queue reaches
                # them.
                q, kd = q_t[d], kd_t[d]
                ah = aps.tile([128, 2048], F32, tag="ah", name="ah")
                inter = dict(interleave)
                for kbp in range(NKB // 2):
                    for fn in inter.get(kbp, ()):
                        fn()
                    grhs = {}
                    for half in range(2):
                        kb = kbp * 2 + half
                        g = gpool.tile([128, 32, 64], BF16, tag="g", name=f"g{half}")
                        # G[k, j, i] = K[k,j] * Q[k,i] (2x-mode paired APs)
                        in0 = bass.AP(
                            tensor=kd.tensor,
                            offset=kd.offset + kb * 128 + jh * 64,
                            ap=[kd.ap[0], [2, 32], [0, 32], [1, 2]])
                        in1 = bass.AP(
                            tensor=q.tensor, offset=q.offset + kb * 64,
                            ap=[q.ap[0], [0, 32], [2, 32], [1, 2]])
                        gout = bass.AP(
                            tensor=g.tensor, offset=g.offset,
                            ap=[g.ap[0], [64, 32], [2, 32], [1, 2]])
                        nc.vector.tensor_mul(out=gout, in0=in0, in1=in1)
                        grhs[half] = g[:].rearrange("p a b -> p (a b)")
                    for ns in range(4):
                        for half in range(2):
                            kb = kbp * 2 + half
                            nc.tensor.matmul(
                                out=ah[half * 64:half * 64 + 64,
                                       ns * 512:(ns + 1) * 512],
                                lhsT=wc_sb[:, kb, :],
                                rhs=grhs[half][:, ns * 512:(ns + 1) * 512],
                                start=(kbp == 0),
                                stop=(kbp == NKB // 2 - 1 and ns == 3),
                                skip_group_check=True,
                                tile_position=(0, half * 64))
                # fold the odd-half partial into the even-half region via an
                # identity matmul, in quarter-column slices so copy and
                # matmul pipeline.  The final chain splits the copies over
                # ACT and DVE (both free once the last G tile is out) so the
                # fold wall-time halves; mid-phase chains keep them on ACT.
                fold = work.tile([128, 2048], BF16, tag="fold", name="fold", bufs=2)
                for ns in range(4):
                    sl = slice(ns * 512, (ns + 1) * 512)
                    if final and ns % 2:
                        with nc.allow_low_precision(reason="fold in bf16"):
                            nc.vector.tensor_copy(out=fold[64:128, sl],
                                                  in_=ah[64:128, sl])
                    else:
                        nc.scalar.copy(out=fold[64:128, sl], in_=ah[64:128, sl])
                    nc.tensor.matmul(
                        out=ah[0:64, sl],
                        lhsT=ident_sb[64:128, :],
                        rhs=fold[64:128, sl],
                        start=False, stop=True,
                        skip_group_check=True,
                        tile_position=(64, 0))
                # exp with transposed read, ah[(j,i)] -> att[(i, j)], in two
                # i-halves so the Z sums of a final chain can start early.
                # post_hs emits each half's Z matmuls right after its exp:
                # consecutive ACT exps otherwise share a batched semaphore
                # and the first half's Z would wait for the second exp too.
                for hs in range(2):
                    src = bass.AP(tensor=ah.tensor, offset=ah.offset + hs * 32,
                                  ap=[[ah.ap[0][0], 64], [1, 32], [64, 32]])
                    nc.scalar.activation(
                        out=att_t[d][:, hs * 32:(hs + 1) * 32,
                                     jh * 32:(jh + 1) * 32], in_=src,
                        func=mybir.ActivationFunctionType.Exp,
                        bias=bcv[:], scale=1.0)
                    if post_hs is not None:
                        post_hs(hs)

            def z_sums(att3, chunks, zs_dst):
                # Z column sums: K=64 ones-matmuls into four disjoint
                # column-groups (psum rows 0/32/64/96) of ONE bank-wide
                # tile, then a single strided DMA spreads them into the
                # [rows, 32/64-wide] zs block for the reciprocal.
                zt4 = cps.tile([128, 512], F32, tag="cv", name="zt4")
                for c4, (off, apf) in enumerate(chunks):
                    rhs = bass.AP(tensor=att3.tensor, offset=att3.offset + off,
                                  ap=[att3.ap[0]] + apf)
                    nc.tensor.matmul(out=zt4[32 * c4:32 * c4 + 1, :],
                                     lhsT=ones[0:64], rhs=rhs,
                                     start=True, stop=True,
                                     skip_group_check=True,
                                     tile_position=(0, 32 * c4))
                zsp = work.tile([128, 512], BF16, tag="zsp", bufs=2, name="zsp")
                with nc.allow_low_precision(reason="Z sums to bf16 for 1/Z"):
                    nc.scalar.copy(out=zsp[:], in_=zt4[:])
                nc.scalar.dma_start(
                    out=zs_dst,
                    in_=bass.AP(tensor=zsp.tensor, offset=zsp.offset,
                                ap=[[zsp.ap[0][0] * 32, 4], [1, 512]]))

            def z_mms_j(d, jh):
                # per-column-half Z sums, (i-major, 32 j) layout
                zs = work.tile([64, 32], BF16, tag="zsj", bufs=2, name="zsj")
                z_sums(att_t[d],
                       [(c4 * 16 * 64 + jh * 32, [[64, 16], [1, 32]])
                        for c4 in range(4)], zs[:])
                return zs

            def rz_chain_j(zs):
                # reciprocal + DRAM-broadcast of 1/Z for one column half
                rzs = work.tile([64, 32], BF16, tag="rzsj", bufs=2, name="rzsj")
                with nc.allow_low_precision(reason="1/Z multiplier in bf16"):
                    nc.vector.reciprocal(out=rzs[:], in_=zs[:])
                rz = dpool.tile([64, 32], BF16, tag="rzdj")
                nc.scalar.dma_start(out=rz[:], in_=rzs[:])
                rzb = work.tile([64, 2048], BF16, tag="rzbj", bufs=2, name="rzbj")
                for qi, queue in enumerate((nc.sync, nc.scalar)):
                    queue.dma_start(
                        out=rzb[:, qi * 1024:(qi + 1) * 1024],
                        in_=bass.AP(tensor=rz.tensor, offset=rz.offset + qi * 1024,
                                    ap=[[0, 64], [32, 32], [1, 32]]))
                return rzb

            # ---- w direction: full-width softmax chain, deferred into the
            # h/jh0 G stream ----------------------------------------------
            g_chain("w", 0)
            g_chain("w", 1)
            att_w = att_t["w"][:].rearrange("p a b -> p (a b)")
            if debug:
                nc.sync.dma_start(out=taps["t_z"][:], in_=att_w[:])
            zs_w = work.tile([64, 64], BF16, tag="zsw", bufs=1)
            for hb in range(2):
                z_sums(att_t["w"],
                       [((hb * 4 + c4) * 512, [[1, 512]]) for c4 in range(4)],
                       zs_w[hb * 32:(hb + 1) * 32, :])
            rzb_w = work.tile([64, N], BF16, tag="rzbw", bufs=1)
            tmp_w = work.tile([64, N], BF16, tag="tmpw", bufs=1)

            def w_recip():
                rzs = work.tile([64, 64], BF16, tag="rzsw", bufs=1)
                with nc.allow_low_precision(reason="1/Z multiplier in bf16"):
                    nc.vector.reciprocal(out=rzs[:], in_=zs_w[:])
                rz = dpool.tile([64, 64], BF16, tag="rzdw")
                nc.scalar.dma_start(out=rz[:], in_=rzs[:])
                for ch in range(2):
                    sl = slice(ch * 2048, (ch + 1) * 2048)
                    nc.sync.dma_start(
                        out=rzb_w[:, sl],
                        in_=bass.AP(tensor=rz.tensor, offset=rz.offset + ch * 2048,
                                    ap=[[0, 64], [64, 32], [1, 64]]))

            def w_tmp():
                nc.vector.tensor_mul(out=tmp_w[:], in0=att_w[:], in1=xs[:])

            def w_hat(ch):
                sl = slice(ch * 2048, (ch + 1) * 2048)
                nc.vector.tensor_mul(out=hat_t["w"][:, sl], in0=tmp_w[:, sl],
                                     in1=rzb_w[:, sl])
                nc.sync.dma_start(
                    out=pad_interior_ap(catA, 64, 128, row0=ch * 32, nrows=32),
                    in_=hat_t["w"][:, sl])

            # ---- h direction, column half 0 ------------------------------
            g_chain("h", 0, {1: [w_recip], 2: [w_tmp],
                             4: [lambda: w_hat(0)], 5: [lambda: w_hat(1)]})
            zs_h0 = z_mms_j("h", 0)
            conv_A(cps, [0, 1, 2])
            conv_A3 = lambda: conv_A(cps, [3])  # noqa: E731
            rzb_h0 = [None]
            tmp_h0 = work.tile([64, 2048], BF16, tag="tmph0", bufs=1)

            def h0_recip():
                rzb_h0[0] = rz_chain_j(zs_h0)

            def h0_tmp():
                in0 = bass.AP(tensor=att_t["h"].tensor, offset=att_t["h"].offset,
                              ap=[att_t["h"].ap[0], [64, 64], [1, 32]])
                in1 = bass.AP(tensor=xs.tensor, offset=xs.offset,
                              ap=[xs.ap[0], [64, 64], [1, 32]])
                nc.vector.tensor_mul(out=tmp_h0[:], in0=in0, in1=in1)

            def h0_hat():
                cb = catB[0:64, :]
                nc.vector.tensor_mul(
                    out=bass.AP(tensor=cb.tensor, offset=cb.offset + PAD + 1,
                                ap=[cb.ap[0], [PAD, 64], [1, 32]]),
                    in0=tmp_h0[:].rearrange("p (a b) -> p a b", b=32),
                    in1=rzb_h0[0][:].rearrange("p (a b) -> p a b", b=32))

            # ---- h direction, column half 1 (the tail) -------------------
            # conv_A rp3 and the left convB column-halves (which only need
            # h0's j<32 attention, ready at slot 5) fill the PE's idle
            # cycles inside this chain's G stream, leaving just the right
            # 33 output columns per rp for the tail.
            g_chain("h", 1, {3: [h0_recip], 4: [h0_tmp], 6: [h0_hat],
                             7: [conv_A3],
                             9: [lambda: conv_B_taps(0, 0, 31, False)],
                             11: [lambda: conv_B_taps(1, 0, 31, False)],
                             13: [lambda: conv_B_taps(2, 0, 31, False)],
                             15: [lambda: conv_B_taps(3, 0, 31, False)]},
                    final=True)
            att_h = att_t["h"][:].rearrange("p a b -> p (a b)")
            if debug:
                nc.sync.dma_start(out=taps["t_att"][:], in_=att_h[:])
            # Tail softmax denominators without DRAM round-trips: Z band
            # matmuls (32-row replication via ones32) fill a [128, 512]
            # psum region -> one ACT Ln (fp16) -> per-chunk (-1)-weights
            # broadcast matmuls put -lnZ on 64 partitions -> ACT exp reads
            # psum, yielding 1/Z in bf16 SBUF for the 2x-mode hat muls.
            # All of zt4n + the four chunks pack into one psum tile (the
            # c4=3 chunk reuses the zt4n columns after Ln has read them);
            # exp/ln/identity share one ACT table (see _patch_act_tables)
            # so no table reloads appear on this chain.
            rzpt = aps.tile([128, 2048], F32, tag="ah", name="rzp")
            zt4n = rzpt[:, 0:512]
            for c4 in range(4):
                rhs = bass.AP(
                    tensor=att_t["h"].tensor,
                    offset=att_t["h"].offset + c4 * 16 * 64 + 32,
                    ap=[att_t["h"].ap[0], [64, 16], [1, 32]])
                nc.tensor.matmul(out=zt4n[32 * c4:32 * c4 + 32, :],
                                 lhsT=ones32[:], rhs=rhs,
                                 start=True, stop=True,
                                 skip_group_check=True,
                                 tile_position=(0, 32 * c4))
            lnz = work.tile([128, 512], F16, tag="lnz", bufs=1, name="lnz")
            with nc.allow_low_precision(reason="lnZ in fp16 (10-bit mantissa)"):
                nc.scalar.activation(out=lnz[:], in_=zt4n,
                                     func=mybir.ActivationFunctionType.Ln,
                                     scale=1.0)
            # all broadcasts BEFORE any exp, packed as column-tile pairs
            # at (0,0)/(0,64).  NOTE: interleaving each exp right after its
            # broadcast looks attractive but RACES on hardware
            # (nondeterministic 5e-2..9e-2 errors) -- keep the batch order.
            dsts = []
            for c4 in range(4):
                dst = rzpt[(c4 % 2) * 64:(c4 % 2) * 64 + 64,
                           (1 + c4 // 2) * 512:(2 + c4 // 2) * 512]
                nc.tensor.matmul(out=dst, lhsT=selneg[:, c4, :], rhs=lnz[:],
                                 start=True, stop=True,
                                 skip_group_check=True,
                                 tile_position=(0, (c4 % 2) * 64))
                dsts.append(dst)
            rzb4 = []
            for c4 in range(4):
                rb = work.tile([64, 512], BF16, tag="rzb4", bufs=4, name="rzb4")
                with nc.allow_low_precision(reason="1/Z multiplier in bf16"):
                    nc.scalar.activation(
                        out=rb[:], in_=dsts[c4],
                        func=mybir.ActivationFunctionType.Exp, scale=1.0)
                rzb4.append(rb)
            # apply in 4 row bands (att*x products first -- they only need
            # the exp -- then the 1/Z muls as the broadcast lands); conv
            # row-pair rp needs image rows up to 16(rp+1)+1, so emit conv
            # rp-1 after each band.
            # att*x products as ONE DVE op (the hats are rb-gated ~3us
            # later, so the coarser granularity costs nothing and saves
            # three instruction + semaphore overheads on the DVE queue)
            tw4 = work.tile([64, 2048], BF16, tag="tmph1", bufs=1, name="tmph1")
            in0 = bass.AP(tensor=att_t["h"].tensor,
                          offset=att_t["h"].offset + 32,
                          ap=[att_t["h"].ap[0], [64, 64], [1, 32]])
            in1 = bass.AP(tensor=xs.tensor, offset=xs.offset + 32,
                          ap=[xs.ap[0], [64, 64], [1, 32]])
            nc.vector.tensor_mul(out=tw4[:], in0=in0, in1=in1)
            for ch in range(4):
                cb = catB[0:64, :]
                nc.vector.tensor_mul(
                    out=bass.AP(tensor=cb.tensor,
                                offset=cb.offset + (ch * 16 + 1) * PAD + 33,
                                ap=[cb.ap[0], [PAD, 16], [1, 32]]),
                    in0=bass.AP(tensor=tw4.tensor,
                                offset=tw4.offset + ch * 512,
                                ap=[tw4.ap[0], [32, 16], [1, 32]]),
                    in1=rzb4[ch][:].rearrange("p (a b) -> p a b", b=32))
                if ch >= 1:
                    conv_B_taps(ch - 1, 31, 33, True)
                    conv_B_fin(ch - 1)
            conv_B_taps(3, 31, 33, True)
            conv_B_fin(3)

        if debug:
            nc.sync.dma_start(out=taps["t_watt"][:], in_=hat_t["w"][:])
            nc.sync.dma_start(out=taps["t_hatt"][:],
                              in_=pad_interior_ap(catB, 0, 64))

    nc.finalize()
    return nc


def _host_prep(Wq, bq, Wk, bk, Wc, bc, Wo, bo, gamma, beta, run_mean, run_var):
    bf = ml_dtypes.bfloat16
    # Wc permuted so the contraction index is (spatial, channel)
    wcp = Wc.reshape(C, C, L).transpose(0, 2, 1).reshape(C, C * L)
    wcpt = np.ascontiguousarray(
        wcp.T.reshape(NKB, 128, 64).transpose(1, 0, 2))  # [128, 32, 64]
    inv = gamma / np.sqrt(run_var + BN_EPS)
    wo_eff = Wo * inv[:, None, None, None]
    wot = wo_eff.transpose(1, 2, 3, 0).reshape(3 * C, 9, C)  # [192, 9, 64]
    # conv image A carries [x; w_att], image B carries h_att plus a ones
    # row whose tap-0 weight injects relu-bias + BN-shift into the psum
    d_vec = beta - run_mean * inv
    brow = np.zeros((1, 9, C), np.float32)
    brow[0, 0, :] = bo * inv + d_vec
    wq2 = np.concatenate([Wq.T, Wq.T])  # [128, 64]
    wk2 = np.concatenate([Wk.T, Wk.T])
    bias3 = np.stack([np.concatenate([bq, bq]), np.concatenate([bk, bk]),
                      np.concatenate([bc, bc])], axis=1)  # [128, 3]
    # -lnZ broadcast weights: selneg[:, c4, :] is -1 at partition 32*c4
    selneg = np.zeros((128, 4, 64), np.float16)
    for c4 in range(4):
        selneg[32 * c4, c4, :] = -1.0
    return {
        "selneg": selneg,
        "wqk": np.ascontiguousarray(
            np.concatenate([wq2, wk2], axis=1)).astype(bf),
        "wcpt": wcpt.astype(bf),
        "woa": np.ascontiguousarray(
            np.concatenate([wot[0:64], wot[128:192]])).astype(bf),
        "wob": np.ascontiguousarray(
            np.concatenate([wot[64:128], brow])).astype(bf),
        "bias3": np.ascontiguousarray(bias3).astype(np.float32),
        "d_vec": d_vec.reshape(64, 1).astype(np.float32),
        "ident": np.concatenate([np.zeros((64, 64), np.float32),
                                 np.eye(64, dtype=np.float32)]).astype(bf),
    }


def kernel(x, Wq, bq, Wk, bk, Wc, bc, Wo, bo, gamma, beta, run_mean, run_var,
           debug=False, trace=False, trace_kwargs=None):
    x = np.asarray(x, np.float32)
    weights = _host_prep(
        np.asarray(Wq, np.float32), np.asarray(bq, np.float32),
        np.asarray(Wk, np.float32), np.asarray(bk, np.float32),
        np.asarray(Wc, np.float32), np.asarray(bc, np.float32),
        np.asarray(Wo, np.float32), np.asarray(bo, np.float32),
        np.asarray(gamma, np.float32), np.asarray(beta, np.float32),
        np.asarray(run_mean, np.float32), np.asarray(run_var, np.float32))
    key = bool(debug)
    if key not in _CACHE:
        _CACHE[key] = _build_nc(debug=debug)
    nc = _CACHE[key]
    bf = ml_dtypes.bfloat16
    in_maps = []
    for b in range(B):
        m = dict(weights)
        xr = x[b].reshape(C, N).astype(bf)
        m["x2bf"] = np.ascontiguousarray(
            np.concatenate([xr[:, 0:N // 2], xr[:, N // 2:]], axis=0))
        in_maps.append(m)
    kwargs = {}
    if trace:
        kwargs = dict(trace=True, trace_cores=[0], **(trace_kwargs or {}))
    res = run_bass_kernel_spmd(nc, in_maps, core_ids=list(range(B)), **kwargs)
    out = np.stack([res.results[b]["y"].astype(np.float32).reshape(C, L, L)
                    for b in range(B)])
    if debug or trace:
        return out, res
    return out

